# revision 7
# baseline (speedup 1.0000x reference)
"""Trainium2 Bass kernel for DLLinearZeroDiagonal:
    y = x @ W.T + bias,  W = zero-diagonal 4096x4096 with strict triangles
    packed row-major in upper_w / lower_w.

Strategy (8 NeuronCores):
  - RO-way shard over output dim x RB-way shard over batch (RO*RB = 8).
  - Host reconstructs the dense weight (sanctioned by the sharding hint:
    "replicate the reconstructed weight") and lays out W^T / x^T shards in
    the tile order the device DMAs want.  All FLOPs + bias happen on device.
  - Per core: resident x^T shard in SBUF, stream W^T slabs once,
    accumulating matmuls (128x128 @ 128xNF), bias add on DVE, outputs
    written as y^T shard and untransposed on host.

Measured per-iteration body time (R=257 on-device repeat loops,
median-differenced, axon trn2):
  fp32r RO2/RB4 NF512 ("resident2"): ~309 us   (fp32r MMs self-load weights
                                                serially: ~300 ns/MM on HW)
  bf16  RO2/RB4 NF512 ("bf16"):      ~273 us   (213 ns fill + ~53 ns LDW)
"""

import os as _os

import numpy as np

N = 4096            # in/out feature dim and batch
NT = N // 128       # 32 contraction tiles

# variant name -> (RO, RB, NF, dtype, opts)
VARIANTS = {
    # output-shard x batch-shard x psum width x input dtype
    "resident2": dict(ro=2, rb=4, nf=512, dt="float32r"),
    "bf16":      dict(ro=2, rb=4, nf=512, dt="bfloat16"),
    "bf16n256":  dict(ro=2, rb=4, nf=256, dt="bfloat16"),
    "bf16nn4":   dict(ro=4, rb=2, nf=512, dt="bfloat16", xchunk=True),
    "bf16nn4c":  dict(ro=4, rb=2, nf=512, dt="bfloat16", xchunk=True,
                      wpair=True),
}

DEFAULT_VARIANT = _os.environ.get("KERNEL_VARIANT", "bf16")

_PROGRAMS = {}


def _cfg(variant=None):
    v = VARIANTS[variant or DEFAULT_VARIANT]
    ro, rb, nf = v["ro"], v["rb"], v["nf"]
    oc, bc = N // ro, N // rb
    return dict(
        ro=ro, rb=rb, nf=nf, dt=v["dt"], oc=oc, bc=bc,
        nw=oc // 128, nn=bc // nf,
        xchunk=v.get("xchunk", False), wpair=v.get("wpair", False),
    )


def _build_program(reps=None, variant=None):
    import concourse.bacc as bacc
    import concourse.bass as bass
    import concourse.tile as tile
    from concourse import mybir
    from contextlib import ExitStack, nullcontext

    c = _cfg(variant)
    OC, BC, NW, NN, NF = c["oc"], c["bc"], c["nw"], c["nn"], c["nf"]
    F32 = mybir.dt.float32
    DT = getattr(mybir.dt, c["dt"])

    nc = bacc.Bacc("TRN2", target_bir_lowering=False, debug=False)
    # host-tiled layouts (see _shard_inputs):
    #   xt[t, p, b]     = x[b0+b, 128t+p]
    #   wt[w, p, t, o'] = W[o0+128w+o', 128t+p]
    #   bias2[p, w]     = bias[o0+128w+p]
    xt = nc.dram_tensor("xt", [NT, 128, BC], DT, kind="ExternalInput")
    wt = nc.dram_tensor("wt", [NW, 128, NT, 128], DT, kind="ExternalInput")
    bias = nc.dram_tensor("bias", [128, NW], F32, kind="ExternalInput")
    yt = nc.dram_tensor("yt", [OC, BC], F32, kind="ExternalOutput")

    with tile.TileContext(nc) as tc, ExitStack() as ctx:
        xtp = ctx.enter_context(tc.tile_pool(name="xtp", bufs=1))
        wtp = ctx.enter_context(tc.tile_pool(name="wtp", bufs=3))
        bp = ctx.enter_context(tc.tile_pool(name="bp", bufs=1))
        op = ctx.enter_context(tc.tile_pool(name="op", bufs=4))
        pp = ctx.enter_context(tc.tile_pool(name="pp", bufs=8, space="PSUM"))

        loop = tc.For_i(0, reps, 1) if reps is not None else nullcontext()
        with loop:
            # resident x^T shard; column block t holds j=128t+p
            if c["xchunk"]:
                xchunks = []
                for t in range(NT):
                    ch = xtp.tile([128, BC], DT, name=f"xc{t}")
                    nc.scalar.dma_start(
                        ch[:], bass.AP(xt, t * 128 * BC, [[BC, 128], [1, BC]]))
                    xchunks.append(ch)
                xv = lambda t, lo, hi: xchunks[t][:, lo:hi]
            else:
                xt_res = xtp.tile([128, NT * BC], DT)
                for t in range(NT):
                    nc.scalar.dma_start(
                        xt_res[:, t * BC:(t + 1) * BC],
                        bass.AP(xt, t * 128 * BC, [[BC, 128], [1, BC]]),
                    )
                xv = lambda t, lo, hi: xt_res[:, t * BC + lo:t * BC + hi]
            bias_sb = bp.tile([128, NW], F32)
            nc.sync.dma_start(bias_sb[:], bass.AP(bias, 0, [[NW, 128], [1, NW]]))

            step = 2 if c["wpair"] else 1
            for w0 in range(0, NW, step):
                ws = list(range(w0, min(w0 + step, NW)))
                slabs, psets = [], []
                for w in ws:
                    slab = wtp.tile([128, NT * 128], DT)
                    nc.sync.dma_start(
                        slab[:],
                        bass.AP(wt, w * 128 * NT * 128,
                                [[NT * 128, 128], [1, NT * 128]]),
                    )
                    slabs.append(slab)
                    psets.append([
                        pp.tile([128, NF], F32, name=f"ps{w % 2}_{n}", tag="ps")
                        for n in range(NN)
                    ])
                for t in range(NT):
                    for slab, psums in zip(slabs, psets):
                        lhsT = slab[:, t * 128:(t + 1) * 128]
                        for n in range(NN):
                            nc.tensor.matmul(
                                psums[n][:],
                                lhsT,
                                xv(t, n * NF, n * NF + NF),
                                start=(t == 0),
                                stop=(t == NT - 1),
                            )
                for w, psums in zip(ws, psets):
                    for n in range(NN):
                        ot = op.tile([128, NF], F32)
                        nc.vector.tensor_scalar_add(ot[:], psums[n][:],
                                                    bias_sb[:, w:w + 1])
                        nc.scalar.dma_start(
                            bass.AP(yt, w * 128 * BC + n * NF,
                                    [[BC, 128], [1, NF]]),
                            ot[:],
                        )
    nc.compile()
    return nc


def _get_program():
    key = DEFAULT_VARIANT
    if key not in _PROGRAMS:
        _PROGRAMS[key] = _build_program()
    return _PROGRAMS[key]


def _reconstruct_wt(upper_w: np.ndarray, lower_w: np.ndarray) -> np.ndarray:
    """Dense W [o, j] from the packed strict triangles (row-major fill)."""
    W = np.zeros((N, N), dtype=np.float32)
    iu = np.triu_indices(N, k=1)
    il = np.tril_indices(N, k=-1)
    W[iu] = upper_w
    W[il] = lower_w
    return W


def _shard_inputs(x, upper_w, lower_w, bias):
    c = _cfg()
    RO, RB, OC, BC, NW = c["ro"], c["rb"], c["oc"], c["bc"], c["nw"]

    x = np.asarray(x, dtype=np.float32)
    upper_w = np.asarray(upper_w, dtype=np.float32)
    lower_w = np.asarray(lower_w, dtype=np.float32)
    bias = np.asarray(bias, dtype=np.float32)

    W = _reconstruct_wt(upper_w, lower_w)

    if c["dt"] == "bfloat16":
        import ml_dtypes
        in_dt = ml_dtypes.bfloat16
    else:
        in_dt = np.float32

    wt_shards = []
    bias_shards = []
    for ob in range(RO):
        Ws = W[ob * OC:(ob + 1) * OC, :]                       # [OC o, N j]
        # wt[w, p, t, o'] = Ws[128w+o', 128t+p]
        wt = np.ascontiguousarray(
            Ws.T.reshape(NT, 128, NW, 128).transpose(2, 1, 0, 3)
        ).astype(in_dt)
        wt_shards.append(wt)
        bias_shards.append(
            np.ascontiguousarray(bias[ob * OC:(ob + 1) * OC].reshape(NW, 128).T)
        )

    xt_shards = []
    for bb in range(RB):
        xs = x[bb * BC:(bb + 1) * BC, :]                       # [BC b, N j]
        xt_shards.append(
            np.ascontiguousarray(xs.T.reshape(NT, 128, BC)).astype(in_dt)
        )

    in_maps = []
    for core in range(8):
        ob, bb = core // RB, core % RB
        in_maps.append({
            "xt": xt_shards[bb],
            "wt": wt_shards[ob],
            "bias": bias_shards[ob],
        })
    return in_maps


def _assemble(results) -> np.ndarray:
    c = _cfg()
    RB, OC, BC = c["rb"], c["oc"], c["bc"]
    y = np.empty((N, N), dtype=np.float32)
    for core in range(8):
        ob, bb = core // RB, core % RB
        y[bb * BC:(bb + 1) * BC, ob * OC:(ob + 1) * OC] = results[core]["yt"].T
    return y


def kernel(x, upper_w, lower_w, bias):
    from concourse import bass_utils

    nc = _get_program()
    in_maps = _shard_inputs(x, upper_w, lower_w, bias)
    res = bass_utils.run_bass_kernel_spmd(nc, in_maps, core_ids=list(range(8)))
    return _assemble(res.results)


# revision 12
# speedup vs baseline: 1.0246x; 1.0246x over previous
"""Trainium2 Bass kernel for DLLinearZeroDiagonal:
    y = x @ W.T + bias,  W = zero-diagonal 4096x4096 with strict triangles
    packed row-major in upper_w / lower_w.

Strategy (8 NeuronCores):
  - RO-way shard over output dim x RB-way shard over batch (RO*RB = 8).
  - Host reconstructs the dense weight (sanctioned by the sharding hint:
    "replicate the reconstructed weight") and lays out W^T / x^T shards in
    the tile order the device DMAs want.  All FLOPs + bias happen on device.
  - Per core: resident x^T shard in SBUF, stream W^T slabs once,
    accumulating matmuls (128x128 @ 128xNF), bias add on DVE, outputs
    written as y^T shard and untransposed on host.

Measured per-iteration body time (R=257 on-device repeat loops,
median-differenced, axon trn2):
  fp32r RO2/RB4 NF512 ("resident2"): ~309 us   (fp32r MMs self-load weights
                                                serially: ~300 ns/MM on HW)
  bf16  RO2/RB4 NF512 ("bf16"):      ~273 us   (213 ns fill + ~53 ns LDW)
"""

import os as _os

import numpy as np

N = 4096            # in/out feature dim and batch
NT = N // 128       # 32 contraction tiles

# variant name -> (RO, RB, NF, dtype, opts)
VARIANTS = {
    # output-shard x batch-shard x psum width x input dtype
    "resident2": dict(ro=2, rb=4, nf=512, dt="float32r"),
    "bf16":      dict(ro=2, rb=4, nf=512, dt="bfloat16"),
    "bf16n256":  dict(ro=2, rb=4, nf=256, dt="bfloat16"),
    "bf16nn4":   dict(ro=4, rb=2, nf=512, dt="bfloat16", xchunk=True),
    "bf16nn4c":  dict(ro=4, rb=2, nf=512, dt="bfloat16", xchunk=True,
                      wpair=True),
    # RO=8/RB=1: each weight block sees all 4096 batch rows -> nn=8 LDW
    # amortization; x streamed in 2 j-chunks with bf16 SBUF accumulation.
    "bc4096":    dict(ro=8, rb=1, nf=512, dt="bfloat16", jc=2),
}

DEFAULT_VARIANT = _os.environ.get("KERNEL_VARIANT", "bf16")

_PROGRAMS = {}


def _cfg(variant=None):
    v = VARIANTS[variant or DEFAULT_VARIANT]
    ro, rb, nf = v["ro"], v["rb"], v["nf"]
    oc, bc = N // ro, N // rb
    return dict(
        ro=ro, rb=rb, nf=nf, dt=v["dt"], oc=oc, bc=bc,
        nw=oc // 128, nn=bc // nf,
        xchunk=v.get("xchunk", False), wpair=v.get("wpair", False),
        jc=v.get("jc", 0),
    )


def _build_program(reps=None, variant=None):
    c = _cfg(variant)
    if c["jc"]:
        return _build_program_jc(reps, c)
    return _build_program_std(reps, c)


def _build_program_jc(reps, c):
    """W-stationary with full-batch moving rows (BC=4096 -> 8 psum banks per
    weight block, one distinct LDWEIGHTS per 8 matmuls).  x is streamed in
    `jc` j-chunks; partial sums accumulate in SBUF as bf16."""
    import concourse.bacc as bacc
    import concourse.bass as bass
    import concourse.tile as tile
    from concourse import mybir
    from contextlib import ExitStack, nullcontext

    OC, BC, NW, NN, NF = c["oc"], c["bc"], c["nw"], c["nn"], c["nf"]
    JC = c["jc"]
    NTC = NT // JC                  # t-tiles per j-chunk
    F32 = mybir.dt.float32
    BF16 = mybir.dt.bfloat16
    DT = getattr(mybir.dt, c["dt"])
    assert NN == 8 and NW * 128 == OC

    nc = bacc.Bacc("TRN2", target_bir_lowering=False, debug=False)
    xt = nc.dram_tensor("xt", [NT, 128, BC], DT, kind="ExternalInput")
    wt = nc.dram_tensor("wt", [NW, 128, NT, 128], DT, kind="ExternalInput")
    bias = nc.dram_tensor("bias", [128, NW], F32, kind="ExternalInput")
    yt = nc.dram_tensor("yt", [OC, BC], F32, kind="ExternalOutput")

    with tile.TileContext(nc) as tc, ExitStack() as ctx:
        xtp = ctx.enter_context(tc.tile_pool(name="xtp", bufs=1))
        wtp = ctx.enter_context(tc.tile_pool(name="wtp", bufs=1))
        bp = ctx.enter_context(tc.tile_pool(name="bp", bufs=1))
        ap = ctx.enter_context(tc.tile_pool(name="ap", bufs=1))
        op = ctx.enter_context(tc.tile_pool(name="op", bufs=4))
        pp = ctx.enter_context(tc.tile_pool(name="pp", bufs=8, space="PSUM"))

        loop = tc.For_i(0, reps, 1) if reps is not None else nullcontext()
        with loop:
            bias_sb = bp.tile([128, NW], F32)
            nc.gpsimd.dma_start(bias_sb[:],
                                bass.AP(bias, 0, [[NW, 128], [1, NW]]))
            # resident W shard: per-w slab [128, NT*128] on the (otherwise
            # idle) gpsimd softDGE ring so the SP/ACT rings are free for x
            slabs = []
            for w in range(NW):
                slab = wtp.tile([128, NT * 128], DT, name=f"wslab{w}")
                nc.gpsimd.dma_start(
                    slab[:],
                    bass.AP(wt, w * 128 * NT * 128,
                            [[NT * 128, 128], [1, NT * 128]]),
                )
                slabs.append(slab)
            # bf16 accumulator for the first j-chunk's partial sums (+bias)
            accs = [[ap.tile([128, NF], BF16, name=f"acc{w}_{n}")
                     for n in range(NN)] for w in range(NW)]

            for jc in range(JC):
                xchunks = []
                for tt in range(NTC):
                    t = jc * NTC + tt
                    ch = xtp.tile([128, BC], DT, name=f"xc{tt}")
                    eng = nc.scalar if tt % 2 == 0 else nc.sync
                    eng.dma_start(
                        ch[:], bass.AP(xt, t * 128 * BC, [[BC, 128], [1, BC]]))
                    xchunks.append(ch)
                for w in range(NW):
                    psums = [pp.tile([128, NF], F32, name=f"ps{n}", tag="ps")
                             for n in range(NN)]
                    for tt in range(NTC):
                        t = jc * NTC + tt
                        lhsT = slabs[w][:, t * 128:(t + 1) * 128]
                        for n in range(NN):
                            nc.tensor.matmul(
                                psums[n][:],
                                lhsT,
                                xchunks[tt][:, n * NF:(n + 1) * NF],
                                start=(tt == 0),
                                stop=(tt == NTC - 1),
                            )
                    for n in range(NN):
                        if jc == 0:
                            nc.vector.tensor_scalar_add(
                                accs[w][n][:], psums[n][:], bias_sb[:, w:w + 1])
                        else:
                            ot = op.tile([128, NF], F32)
                            nc.vector.tensor_add(ot[:], psums[n][:],
                                                 accs[w][n][:])
                            eng = nc.scalar if n % 2 == 0 else nc.sync
                            eng.dma_start(
                                bass.AP(yt, w * 128 * BC + n * NF,
                                        [[BC, 128], [1, NF]]),
                                ot[:],
                            )
    nc.compile()
    return nc


def _build_program_std(reps, c):
    import concourse.bacc as bacc
    import concourse.bass as bass
    import concourse.tile as tile
    from concourse import mybir
    from contextlib import ExitStack, nullcontext
    OC, BC, NW, NN, NF = c["oc"], c["bc"], c["nw"], c["nn"], c["nf"]
    F32 = mybir.dt.float32
    DT = getattr(mybir.dt, c["dt"])

    nc = bacc.Bacc("TRN2", target_bir_lowering=False, debug=False)
    # host-tiled layouts (see _shard_inputs):
    #   xt[t, p, b]     = x[b0+b, 128t+p]
    #   wt[w, p, t, o'] = W[o0+128w+o', 128t+p]
    #   bias2[p, w]     = bias[o0+128w+p]
    xt = nc.dram_tensor("xt", [NT, 128, BC], DT, kind="ExternalInput")
    wt = nc.dram_tensor("wt", [NW, 128, NT, 128], DT, kind="ExternalInput")
    bias = nc.dram_tensor("bias", [128, NW], F32, kind="ExternalInput")
    yt = nc.dram_tensor("yt", [OC, BC], F32, kind="ExternalOutput")

    with tile.TileContext(nc) as tc, ExitStack() as ctx:
        xtp = ctx.enter_context(tc.tile_pool(name="xtp", bufs=1))
        wtp = ctx.enter_context(tc.tile_pool(name="wtp", bufs=3))
        bp = ctx.enter_context(tc.tile_pool(name="bp", bufs=1))
        op = ctx.enter_context(tc.tile_pool(name="op", bufs=4))
        pp = ctx.enter_context(tc.tile_pool(name="pp", bufs=8, space="PSUM"))

        loop = tc.For_i(0, reps, 1) if reps is not None else nullcontext()
        with loop:
            # resident x^T shard; column block t holds j=128t+p
            if c["xchunk"]:
                xchunks = []
                for t in range(NT):
                    ch = xtp.tile([128, BC], DT, name=f"xc{t}")
                    nc.scalar.dma_start(
                        ch[:], bass.AP(xt, t * 128 * BC, [[BC, 128], [1, BC]]))
                    xchunks.append(ch)
                xv = lambda t, lo, hi: xchunks[t][:, lo:hi]
            else:
                xt_res = xtp.tile([128, NT * BC], DT)
                for t in range(NT):
                    nc.scalar.dma_start(
                        xt_res[:, t * BC:(t + 1) * BC],
                        bass.AP(xt, t * 128 * BC, [[BC, 128], [1, BC]]),
                    )
                xv = lambda t, lo, hi: xt_res[:, t * BC + lo:t * BC + hi]
            bias_sb = bp.tile([128, NW], F32)
            nc.sync.dma_start(bias_sb[:], bass.AP(bias, 0, [[NW, 128], [1, NW]]))

            step = 2 if c["wpair"] else 1
            for w0 in range(0, NW, step):
                ws = list(range(w0, min(w0 + step, NW)))
                slabs, psets = [], []
                for w in ws:
                    slab = wtp.tile([128, NT * 128], DT)
                    nc.sync.dma_start(
                        slab[:],
                        bass.AP(wt, w * 128 * NT * 128,
                                [[NT * 128, 128], [1, NT * 128]]),
                    )
                    slabs.append(slab)
                    psets.append([
                        pp.tile([128, NF], F32, name=f"ps{w % 2}_{n}", tag="ps")
                        for n in range(NN)
                    ])
                for t in range(NT):
                    for slab, psums in zip(slabs, psets):
                        lhsT = slab[:, t * 128:(t + 1) * 128]
                        for n in range(NN):
                            nc.tensor.matmul(
                                psums[n][:],
                                lhsT,
                                xv(t, n * NF, n * NF + NF),
                                start=(t == 0),
                                stop=(t == NT - 1),
                            )
                for w, psums in zip(ws, psets):
                    for n in range(NN):
                        ot = op.tile([128, NF], F32)
                        nc.vector.tensor_scalar_add(ot[:], psums[n][:],
                                                    bias_sb[:, w:w + 1])
                        nc.scalar.dma_start(
                            bass.AP(yt, w * 128 * BC + n * NF,
                                    [[BC, 128], [1, NF]]),
                            ot[:],
                        )
    nc.compile()
    return nc


def _get_program():
    key = DEFAULT_VARIANT
    if key not in _PROGRAMS:
        _PROGRAMS[key] = _build_program()
    return _PROGRAMS[key]


def _reconstruct_wt(upper_w: np.ndarray, lower_w: np.ndarray) -> np.ndarray:
    """Dense W [o, j] from the packed strict triangles (row-major fill)."""
    W = np.zeros((N, N), dtype=np.float32)
    iu = np.triu_indices(N, k=1)
    il = np.tril_indices(N, k=-1)
    W[iu] = upper_w
    W[il] = lower_w
    return W


def _shard_inputs(x, upper_w, lower_w, bias):
    c = _cfg()
    RO, RB, OC, BC, NW = c["ro"], c["rb"], c["oc"], c["bc"], c["nw"]

    x = np.asarray(x, dtype=np.float32)
    upper_w = np.asarray(upper_w, dtype=np.float32)
    lower_w = np.asarray(lower_w, dtype=np.float32)
    bias = np.asarray(bias, dtype=np.float32)

    W = _reconstruct_wt(upper_w, lower_w)

    if c["dt"] == "bfloat16":
        import ml_dtypes
        in_dt = ml_dtypes.bfloat16
    else:
        in_dt = np.float32

    wt_shards = []
    bias_shards = []
    for ob in range(RO):
        Ws = W[ob * OC:(ob + 1) * OC, :]                       # [OC o, N j]
        # wt[w, p, t, o'] = Ws[128w+o', 128t+p]
        wt = np.ascontiguousarray(
            Ws.T.reshape(NT, 128, NW, 128).transpose(2, 1, 0, 3)
        ).astype(in_dt)
        wt_shards.append(wt)
        bias_shards.append(
            np.ascontiguousarray(bias[ob * OC:(ob + 1) * OC].reshape(NW, 128).T)
        )

    xt_shards = []
    for bb in range(RB):
        xs = x[bb * BC:(bb + 1) * BC, :]                       # [BC b, N j]
        xt_shards.append(
            np.ascontiguousarray(xs.T.reshape(NT, 128, BC)).astype(in_dt)
        )

    in_maps = []
    for core in range(8):
        ob, bb = core // RB, core % RB
        in_maps.append({
            "xt": xt_shards[bb],
            "wt": wt_shards[ob],
            "bias": bias_shards[ob],
        })
    return in_maps


def _assemble(results) -> np.ndarray:
    c = _cfg()
    RB, OC, BC = c["rb"], c["oc"], c["bc"]
    y = np.empty((N, N), dtype=np.float32)
    for core in range(8):
        ob, bb = core // RB, core % RB
        y[bb * BC:(bb + 1) * BC, ob * OC:(ob + 1) * OC] = results[core]["yt"].T
    return y


def kernel(x, upper_w, lower_w, bias):
    from concourse import bass_utils

    nc = _get_program()
    in_maps = _shard_inputs(x, upper_w, lower_w, bias)
    res = bass_utils.run_bass_kernel_spmd(nc, in_maps, core_ids=list(range(8)))
    return _assemble(res.results)


# revision 14
# speedup vs baseline: 1.1093x; 1.0827x over previous
"""Trainium2 Bass kernel for DLLinearZeroDiagonal:
    y = x @ W.T + bias,  W = zero-diagonal 4096x4096 with strict triangles
    packed row-major in upper_w / lower_w.

Strategy (8 NeuronCores):
  - RO-way shard over output dim x RB-way shard over batch (RO*RB = 8).
  - Host reconstructs the dense weight (sanctioned by the sharding hint:
    "replicate the reconstructed weight") and lays out W^T / x^T shards in
    the tile order the device DMAs want.  All FLOPs + bias happen on device.
  - Per core: resident x^T shard in SBUF, stream W^T slabs once,
    accumulating matmuls (128x128 @ 128xNF), bias add on DVE, outputs
    written as y^T shard and untransposed on host.

Measured per-iteration body time (R=257 on-device repeat loops,
median-differenced, axon trn2) and the per-matmul cost model that fits all
of them:  per-MM = NF/2.4GHz (fill, 1 col/cycle)
                   + 90ns/nn (one serial LDWEIGHTS per distinct weight
                              block, amortized over the nn moving tiles)
                   + 8ns    (issue/semaphore overhead)
  fp32r RO2/RB4 NF512 ("resident2"): ~309 us  (fp32r MMs self-load weights
                                               serially: ~300 ns/MM on HW)
  bf16  RO2/RB4 NF512 ("bf16"):      ~273 us  (266 ns/MM: 213+45+8)
  bf16  RO2/RB4 NF256 ("bf16n256"):  ~286 us  (140 ns/MM x 2048 MMs)
  bf16  RO4/RB2 NF512 ("bf16nn4"):   ~250 us  (244 ns/MM: 213+22.5+8)
  bf16  RO8/RB1 NF512 ("bc4096"):    noisy ~285-310 us session; no win
The bf16 fill floor (524288 PE rows @ 2.4 GHz = 218.5 us/core) makes
~227 us the structural limit; fp8 DoubleRow would halve fill but fails
the 2e-2 gate (pure fp8 = 3.7% rel err; residual-corrected needs 3 MMs).
"""

import os as _os

import numpy as np

N = 4096            # in/out feature dim and batch
NT = N // 128       # 32 contraction tiles

# variant name -> (RO, RB, NF, dtype, opts)
VARIANTS = {
    # output-shard x batch-shard x psum width x input dtype
    "resident2": dict(ro=2, rb=4, nf=512, dt="float32r"),
    "bf16":      dict(ro=2, rb=4, nf=512, dt="bfloat16"),
    "bf16n256":  dict(ro=2, rb=4, nf=256, dt="bfloat16"),
    "bf16nn4":   dict(ro=4, rb=2, nf=512, dt="bfloat16", xchunk=True),
    "bf16nn4c":  dict(ro=4, rb=2, nf=512, dt="bfloat16", xchunk=True,
                      wpair=True),
    # RO=8/RB=1: each weight block sees all 4096 batch rows -> nn=8 LDW
    # amortization; x streamed in 2 j-chunks with bf16 SBUF accumulation.
    "bc4096":    dict(ro=8, rb=1, nf=512, dt="bfloat16", jc=2),
}

DEFAULT_VARIANT = _os.environ.get("KERNEL_VARIANT", "bf16nn4")

_PROGRAMS = {}


def _cfg(variant=None):
    v = VARIANTS[variant or DEFAULT_VARIANT]
    ro, rb, nf = v["ro"], v["rb"], v["nf"]
    oc, bc = N // ro, N // rb
    return dict(
        ro=ro, rb=rb, nf=nf, dt=v["dt"], oc=oc, bc=bc,
        nw=oc // 128, nn=bc // nf,
        xchunk=v.get("xchunk", False), wpair=v.get("wpair", False),
        jc=v.get("jc", 0),
    )


def _build_program(reps=None, variant=None):
    c = _cfg(variant)
    if c["jc"]:
        return _build_program_jc(reps, c)
    return _build_program_std(reps, c)


def _build_program_jc(reps, c):
    """W-stationary with full-batch moving rows (BC=4096 -> 8 psum banks per
    weight block, one distinct LDWEIGHTS per 8 matmuls).  x is streamed in
    `jc` j-chunks; partial sums accumulate in SBUF as bf16."""
    import concourse.bacc as bacc
    import concourse.bass as bass
    import concourse.tile as tile
    from concourse import mybir
    from contextlib import ExitStack, nullcontext

    OC, BC, NW, NN, NF = c["oc"], c["bc"], c["nw"], c["nn"], c["nf"]
    JC = c["jc"]
    NTC = NT // JC                  # t-tiles per j-chunk
    F32 = mybir.dt.float32
    BF16 = mybir.dt.bfloat16
    DT = getattr(mybir.dt, c["dt"])
    assert NN == 8 and NW * 128 == OC

    nc = bacc.Bacc("TRN2", target_bir_lowering=False, debug=False)
    xt = nc.dram_tensor("xt", [NT, 128, BC], DT, kind="ExternalInput")
    wt = nc.dram_tensor("wt", [NW, 128, NT, 128], DT, kind="ExternalInput")
    bias = nc.dram_tensor("bias", [128, NW], F32, kind="ExternalInput")
    yt = nc.dram_tensor("yt", [OC, BC], F32, kind="ExternalOutput")

    with tile.TileContext(nc) as tc, ExitStack() as ctx:
        xtp = ctx.enter_context(tc.tile_pool(name="xtp", bufs=1))
        wtp = ctx.enter_context(tc.tile_pool(name="wtp", bufs=1))
        bp = ctx.enter_context(tc.tile_pool(name="bp", bufs=1))
        ap = ctx.enter_context(tc.tile_pool(name="ap", bufs=1))
        op = ctx.enter_context(tc.tile_pool(name="op", bufs=4))
        pp = ctx.enter_context(tc.tile_pool(name="pp", bufs=8, space="PSUM"))

        loop = tc.For_i(0, reps, 1) if reps is not None else nullcontext()
        with loop:
            bias_sb = bp.tile([128, NW], F32)
            nc.gpsimd.dma_start(bias_sb[:],
                                bass.AP(bias, 0, [[NW, 128], [1, NW]]))
            # resident W shard: per-w slab [128, NT*128] on the (otherwise
            # idle) gpsimd softDGE ring so the SP/ACT rings are free for x
            slabs = []
            for w in range(NW):
                slab = wtp.tile([128, NT * 128], DT, name=f"wslab{w}")
                nc.gpsimd.dma_start(
                    slab[:],
                    bass.AP(wt, w * 128 * NT * 128,
                            [[NT * 128, 128], [1, NT * 128]]),
                )
                slabs.append(slab)
            # bf16 accumulator for the first j-chunk's partial sums (+bias)
            accs = [[ap.tile([128, NF], BF16, name=f"acc{w}_{n}")
                     for n in range(NN)] for w in range(NW)]

            for jc in range(JC):
                xchunks = []
                for tt in range(NTC):
                    t = jc * NTC + tt
                    ch = xtp.tile([128, BC], DT, name=f"xc{tt}")
                    eng = nc.scalar if tt % 2 == 0 else nc.sync
                    eng.dma_start(
                        ch[:], bass.AP(xt, t * 128 * BC, [[BC, 128], [1, BC]]))
                    xchunks.append(ch)
                for w in range(NW):
                    psums = [pp.tile([128, NF], F32, name=f"ps{n}", tag="ps")
                             for n in range(NN)]
                    for tt in range(NTC):
                        t = jc * NTC + tt
                        lhsT = slabs[w][:, t * 128:(t + 1) * 128]
                        for n in range(NN):
                            nc.tensor.matmul(
                                psums[n][:],
                                lhsT,
                                xchunks[tt][:, n * NF:(n + 1) * NF],
                                start=(tt == 0),
                                stop=(tt == NTC - 1),
                            )
                    for n in range(NN):
                        if jc == 0:
                            nc.vector.tensor_scalar_add(
                                accs[w][n][:], psums[n][:], bias_sb[:, w:w + 1])
                        else:
                            ot = op.tile([128, NF], F32)
                            nc.vector.tensor_add(ot[:], psums[n][:],
                                                 accs[w][n][:])
                            eng = nc.scalar if n % 2 == 0 else nc.sync
                            eng.dma_start(
                                bass.AP(yt, w * 128 * BC + n * NF,
                                        [[BC, 128], [1, NF]]),
                                ot[:],
                            )
    nc.compile()
    return nc


def _build_program_std(reps, c):
    import concourse.bacc as bacc
    import concourse.bass as bass
    import concourse.tile as tile
    from concourse import mybir
    from contextlib import ExitStack, nullcontext
    OC, BC, NW, NN, NF = c["oc"], c["bc"], c["nw"], c["nn"], c["nf"]
    F32 = mybir.dt.float32
    DT = getattr(mybir.dt, c["dt"])

    nc = bacc.Bacc("TRN2", target_bir_lowering=False, debug=False)
    # host-tiled layouts (see _shard_inputs):
    #   xt[t, p, b]     = x[b0+b, 128t+p]
    #   wt[w, p, t, o'] = W[o0+128w+o', 128t+p]
    #   bias2[p, w]     = bias[o0+128w+p]
    xt = nc.dram_tensor("xt", [NT, 128, BC], DT, kind="ExternalInput")
    wt = nc.dram_tensor("wt", [NW, 128, NT, 128], DT, kind="ExternalInput")
    bias = nc.dram_tensor("bias", [128, NW], F32, kind="ExternalInput")
    yt = nc.dram_tensor("yt", [OC, BC], F32, kind="ExternalOutput")

    with tile.TileContext(nc) as tc, ExitStack() as ctx:
        xtp = ctx.enter_context(tc.tile_pool(name="xtp", bufs=1))
        wtp = ctx.enter_context(tc.tile_pool(name="wtp", bufs=3))
        bp = ctx.enter_context(tc.tile_pool(name="bp", bufs=1))
        op = ctx.enter_context(tc.tile_pool(name="op", bufs=4))
        pp = ctx.enter_context(tc.tile_pool(name="pp", bufs=8, space="PSUM"))

        loop = tc.For_i(0, reps, 1) if reps is not None else nullcontext()
        with loop:
            # resident x^T shard; column block t holds j=128t+p
            if c["xchunk"]:
                xchunks = []
                for t in range(NT):
                    ch = xtp.tile([128, BC], DT, name=f"xc{t}")
                    nc.scalar.dma_start(
                        ch[:], bass.AP(xt, t * 128 * BC, [[BC, 128], [1, BC]]))
                    xchunks.append(ch)
                xv = lambda t, lo, hi: xchunks[t][:, lo:hi]
            else:
                xt_res = xtp.tile([128, NT * BC], DT)
                for t in range(NT):
                    nc.scalar.dma_start(
                        xt_res[:, t * BC:(t + 1) * BC],
                        bass.AP(xt, t * 128 * BC, [[BC, 128], [1, BC]]),
                    )
                xv = lambda t, lo, hi: xt_res[:, t * BC + lo:t * BC + hi]
            bias_sb = bp.tile([128, NW], F32)
            nc.sync.dma_start(bias_sb[:], bass.AP(bias, 0, [[NW, 128], [1, NW]]))

            step = 2 if c["wpair"] else 1
            for w0 in range(0, NW, step):
                ws = list(range(w0, min(w0 + step, NW)))
                slabs, psets = [], []
                for w in ws:
                    slab = wtp.tile([128, NT * 128], DT)
                    nc.sync.dma_start(
                        slab[:],
                        bass.AP(wt, w * 128 * NT * 128,
                                [[NT * 128, 128], [1, NT * 128]]),
                    )
                    slabs.append(slab)
                    psets.append([
                        pp.tile([128, NF], F32, name=f"ps{w % 2}_{n}", tag="ps")
                        for n in range(NN)
                    ])
                for t in range(NT):
                    for slab, psums in zip(slabs, psets):
                        lhsT = slab[:, t * 128:(t + 1) * 128]
                        for n in range(NN):
                            nc.tensor.matmul(
                                psums[n][:],
                                lhsT,
                                xv(t, n * NF, n * NF + NF),
                                start=(t == 0),
                                stop=(t == NT - 1),
                            )
                for w, psums in zip(ws, psets):
                    for n in range(NN):
                        ot = op.tile([128, NF], F32)
                        nc.vector.tensor_scalar_add(ot[:], psums[n][:],
                                                    bias_sb[:, w:w + 1])
                        nc.scalar.dma_start(
                            bass.AP(yt, w * 128 * BC + n * NF,
                                    [[BC, 128], [1, NF]]),
                            ot[:],
                        )
    nc.compile()
    return nc


def _get_program():
    key = DEFAULT_VARIANT
    if key not in _PROGRAMS:
        _PROGRAMS[key] = _build_program()
    return _PROGRAMS[key]


def _reconstruct_wt(upper_w: np.ndarray, lower_w: np.ndarray) -> np.ndarray:
    """Dense W [o, j] from the packed strict triangles (row-major fill)."""
    W = np.zeros((N, N), dtype=np.float32)
    iu = np.triu_indices(N, k=1)
    il = np.tril_indices(N, k=-1)
    W[iu] = upper_w
    W[il] = lower_w
    return W


def _shard_inputs(x, upper_w, lower_w, bias):
    c = _cfg()
    RO, RB, OC, BC, NW = c["ro"], c["rb"], c["oc"], c["bc"], c["nw"]

    x = np.asarray(x, dtype=np.float32)
    upper_w = np.asarray(upper_w, dtype=np.float32)
    lower_w = np.asarray(lower_w, dtype=np.float32)
    bias = np.asarray(bias, dtype=np.float32)

    W = _reconstruct_wt(upper_w, lower_w)

    if c["dt"] == "bfloat16":
        import ml_dtypes
        in_dt = ml_dtypes.bfloat16
    else:
        in_dt = np.float32

    wt_shards = []
    bias_shards = []
    for ob in range(RO):
        Ws = W[ob * OC:(ob + 1) * OC, :]                       # [OC o, N j]
        # wt[w, p, t, o'] = Ws[128w+o', 128t+p]
        wt = np.ascontiguousarray(
            Ws.T.reshape(NT, 128, NW, 128).transpose(2, 1, 0, 3)
        ).astype(in_dt)
        wt_shards.append(wt)
        bias_shards.append(
            np.ascontiguousarray(bias[ob * OC:(ob + 1) * OC].reshape(NW, 128).T)
        )

    xt_shards = []
    for bb in range(RB):
        xs = x[bb * BC:(bb + 1) * BC, :]                       # [BC b, N j]
        xt_shards.append(
            np.ascontiguousarray(xs.T.reshape(NT, 128, BC)).astype(in_dt)
        )

    in_maps = []
    for core in range(8):
        ob, bb = core // RB, core % RB
        in_maps.append({
            "xt": xt_shards[bb],
            "wt": wt_shards[ob],
            "bias": bias_shards[ob],
        })
    return in_maps


def _assemble(results) -> np.ndarray:
    c = _cfg()
    RB, OC, BC = c["rb"], c["oc"], c["bc"]
    y = np.empty((N, N), dtype=np.float32)
    for core in range(8):
        ob, bb = core // RB, core % RB
        y[bb * BC:(bb + 1) * BC, ob * OC:(ob + 1) * OC] = results[core]["yt"].T
    return y


def kernel(x, upper_w, lower_w, bias):
    from concourse import bass_utils

    nc = _get_program()
    in_maps = _shard_inputs(x, upper_w, lower_w, bias)
    res = bass_utils.run_bass_kernel_spmd(nc, in_maps, core_ids=list(range(8)))
    return _assemble(res.results)


# revision 20
# speedup vs baseline: 1.1268x; 1.0157x over previous
"""Trainium2 Bass kernel for DLLinearZeroDiagonal:
    y = x @ W.T + bias,  W = zero-diagonal 4096x4096 with strict triangles
    packed row-major in upper_w / lower_w.

Strategy (8 NeuronCores):
  - RO-way shard over output dim x RB-way shard over batch (RO*RB = 8).
  - Host reconstructs the dense weight (sanctioned by the sharding hint:
    "replicate the reconstructed weight") and lays out W^T / x^T shards in
    the tile order the device DMAs want.  All FLOPs + bias happen on device.
  - Per core: resident x^T shard in SBUF, stream W^T slabs once,
    accumulating matmuls (128x128 @ 128xNF), bias add on DVE, outputs
    written as y^T shard and untransposed on host.

Measured per-iteration body time (R=257 on-device repeat loops,
median-differenced, axon trn2) and the per-matmul cost model that fits all
of them:  per-MM = NF/2.4GHz (fill, 1 col/cycle)
                   + 90ns/nn (one serial LDWEIGHTS per distinct weight
                              block, amortized over the nn moving tiles)
                   + 8ns    (issue/semaphore overhead)
  fp32r RO2/RB4 NF512 ("resident2"): ~309 us  (fp32r MMs self-load weights
                                               serially: ~300 ns/MM on HW)
  bf16  RO2/RB4 NF512 ("bf16"):      ~273 us  (266 ns/MM: 213+45+8)
  bf16  RO2/RB4 NF256 ("bf16n256"):  ~286 us  (140 ns/MM x 2048 MMs)
  bf16  RO4/RB2 NF512 ("bf16nn4"):   ~250 us  (244 ns/MM: 213+22.5+8)
  bf16  RO8/RB1 NF512 ("bc4096"):    loses ~20-45 us to bf16nn4 in a
      matched-session A/B despite halving the LDW term on paper — the
      x-streaming/softDGE/SBUF-accum structure costs more on HW than
      CoreSim models.  Keep bf16nn4.
The bf16 fill floor (524288 PE rows @ 2.4 GHz = 218.5 us/core) makes
~227 us the structural limit; fp8 DoubleRow would halve fill but fails
the 2e-2 gate (pure fp8 = 3.7% rel err; residual-corrected needs 3 MMs).
NOTE: the device sustains ~2.4 GHz in some sessions and ~2.1-2.2 GHz in
others, so identical kernels measure 250-287 us across sessions; compare
variants only within one session (ab.py).
"""

import os as _os

import numpy as np

N = 4096            # in/out feature dim and batch
NT = N // 128       # 32 contraction tiles

# variant name -> (RO, RB, NF, dtype, opts)
VARIANTS = {
    # output-shard x batch-shard x psum width x input dtype
    "resident2": dict(ro=2, rb=4, nf=512, dt="float32r"),
    "bf16":      dict(ro=2, rb=4, nf=512, dt="bfloat16"),
    "bf16n256":  dict(ro=2, rb=4, nf=256, dt="bfloat16"),
    "bf16nn4":   dict(ro=4, rb=2, nf=512, dt="bfloat16", xchunk=True),
    "bf16nn4c":  dict(ro=4, rb=2, nf=512, dt="bfloat16", xchunk=True,
                      wpair=True),
    # RO=8/RB=1: each weight block sees all 4096 batch rows -> nn=8 LDW
    # amortization; x streamed in 2 j-chunks with bf16 SBUF accumulation.
    "bc4096":    dict(ro=8, rb=1, nf=512, dt="bfloat16", jc=2),
    # nn4 with a single resident x tile (fewer dependency sems than 32
    # chunks) and y-out alternating over both hardware rings.  Matched-
    # session A/B vs bf16nn4: indistinguishable (within +-3 us) — sem
    # granularity and ring balance are sub-noise here.
    "bf16nn4f":  dict(ro=4, rb=2, nf=512, dt="bfloat16", ysplit=True),
}

DEFAULT_VARIANT = _os.environ.get("KERNEL_VARIANT", "bf16nn4")

_PROGRAMS = {}


def _cfg(variant=None):
    v = VARIANTS[variant or DEFAULT_VARIANT]
    ro, rb, nf = v["ro"], v["rb"], v["nf"]
    oc, bc = N // ro, N // rb
    return dict(
        ro=ro, rb=rb, nf=nf, dt=v["dt"], oc=oc, bc=bc,
        nw=oc // 128, nn=bc // nf,
        xchunk=v.get("xchunk", False), wpair=v.get("wpair", False),
        jc=v.get("jc", 0), ysplit=v.get("ysplit", False),
    )


def _build_program(reps=None, variant=None):
    c = _cfg(variant)
    if c["jc"]:
        return _build_program_jc(reps, c)
    return _build_program_std(reps, c)


def _build_program_jc(reps, c):
    """W-stationary with full-batch moving rows (BC=4096 -> 8 psum banks per
    weight block, one distinct LDWEIGHTS per 8 matmuls).  x is streamed in
    `jc` j-chunks; partial sums accumulate in SBUF as bf16."""
    import concourse.bacc as bacc
    import concourse.bass as bass
    import concourse.tile as tile
    from concourse import mybir
    from contextlib import ExitStack, nullcontext

    OC, BC, NW, NN, NF = c["oc"], c["bc"], c["nw"], c["nn"], c["nf"]
    JC = c["jc"]
    NTC = NT // JC                  # t-tiles per j-chunk
    F32 = mybir.dt.float32
    BF16 = mybir.dt.bfloat16
    DT = getattr(mybir.dt, c["dt"])
    assert NN == 8 and NW * 128 == OC

    nc = bacc.Bacc("TRN2", target_bir_lowering=False, debug=False)
    xt = nc.dram_tensor("xt", [NT, 128, BC], DT, kind="ExternalInput")
    wt = nc.dram_tensor("wt", [NW, 128, NT, 128], DT, kind="ExternalInput")
    bias = nc.dram_tensor("bias", [128, NW], F32, kind="ExternalInput")
    yt = nc.dram_tensor("yt", [OC, BC], F32, kind="ExternalOutput")

    with tile.TileContext(nc) as tc, ExitStack() as ctx:
        xtp = ctx.enter_context(tc.tile_pool(name="xtp", bufs=1))
        wtp = ctx.enter_context(tc.tile_pool(name="wtp", bufs=1))
        bp = ctx.enter_context(tc.tile_pool(name="bp", bufs=1))
        ap = ctx.enter_context(tc.tile_pool(name="ap", bufs=1))
        op = ctx.enter_context(tc.tile_pool(name="op", bufs=4))
        pp = ctx.enter_context(tc.tile_pool(name="pp", bufs=8, space="PSUM"))

        loop = tc.For_i(0, reps, 1) if reps is not None else nullcontext()
        with loop:
            bias_sb = bp.tile([128, NW], F32)
            nc.gpsimd.dma_start(bias_sb[:],
                                bass.AP(bias, 0, [[NW, 128], [1, NW]]))
            # resident W shard: per-w slab [128, NT*128].  slab0 gates the
            # first matmul -> SP hardware ring, emitted ahead of the x
            # chunks; slabs 1..NW-1 are needed only w*27us into the
            # iteration -> idle gpsimd softDGE ring (latency-tolerant).
            slabs = []
            for w in range(NW):
                slab = wtp.tile([128, NT * 128], DT, name=f"wslab{w}")
                eng = nc.sync if w == 0 else nc.gpsimd
                eng.dma_start(
                    slab[:],
                    bass.AP(wt, w * 128 * NT * 128,
                            [[NT * 128, 128], [1, NT * 128]]),
                )
                slabs.append(slab)
            # bf16 accumulator for the first j-chunk's partial sums (+bias)
            accs = [[ap.tile([128, NF], BF16, name=f"acc{w}_{n}")
                     for n in range(NN)] for w in range(NW)]

            for jc in range(JC):
                xchunks = []
                for tt in range(NTC):
                    t = jc * NTC + tt
                    ch = xtp.tile([128, BC], DT, name=f"xc{tt}")
                    eng = nc.scalar if tt % 2 == 0 else nc.sync
                    eng.dma_start(
                        ch[:], bass.AP(xt, t * 128 * BC, [[BC, 128], [1, BC]]))
                    xchunks.append(ch)
                for w in range(NW):
                    psums = [pp.tile([128, NF], F32, name=f"ps{n}", tag="ps")
                             for n in range(NN)]
                    for tt in range(NTC):
                        t = jc * NTC + tt
                        lhsT = slabs[w][:, t * 128:(t + 1) * 128]
                        for n in range(NN):
                            nc.tensor.matmul(
                                psums[n][:],
                                lhsT,
                                xchunks[tt][:, n * NF:(n + 1) * NF],
                                start=(tt == 0),
                                stop=(tt == NTC - 1),
                            )
                    for n in range(NN):
                        if jc == 0:
                            nc.vector.tensor_scalar_add(
                                accs[w][n][:], psums[n][:], bias_sb[:, w:w + 1])
                        else:
                            ot = op.tile([128, NF], F32)
                            nc.vector.tensor_add(ot[:], psums[n][:],
                                                 accs[w][n][:])
                            eng = nc.scalar if n % 2 == 0 else nc.sync
                            eng.dma_start(
                                bass.AP(yt, w * 128 * BC + n * NF,
                                        [[BC, 128], [1, NF]]),
                                ot[:],
                            )
    nc.compile()
    return nc


def _build_program_std(reps, c):
    import concourse.bacc as bacc
    import concourse.bass as bass
    import concourse.tile as tile
    from concourse import mybir
    from contextlib import ExitStack, nullcontext
    OC, BC, NW, NN, NF = c["oc"], c["bc"], c["nw"], c["nn"], c["nf"]
    F32 = mybir.dt.float32
    DT = getattr(mybir.dt, c["dt"])

    nc = bacc.Bacc("TRN2", target_bir_lowering=False, debug=False)
    # host-tiled layouts (see _shard_inputs):
    #   xt[t, p, b]     = x[b0+b, 128t+p]
    #   wt[w, p, t, o'] = W[o0+128w+o', 128t+p]
    #   bias2[p, w]     = bias[o0+128w+p]
    xt = nc.dram_tensor("xt", [NT, 128, BC], DT, kind="ExternalInput")
    wt = nc.dram_tensor("wt", [NW, 128, NT, 128], DT, kind="ExternalInput")
    bias = nc.dram_tensor("bias", [128, NW], F32, kind="ExternalInput")
    yt = nc.dram_tensor("yt", [OC, BC], F32, kind="ExternalOutput")

    with tile.TileContext(nc) as tc, ExitStack() as ctx:
        xtp = ctx.enter_context(tc.tile_pool(name="xtp", bufs=1))
        wtp = ctx.enter_context(tc.tile_pool(name="wtp", bufs=3))
        bp = ctx.enter_context(tc.tile_pool(name="bp", bufs=1))
        op = ctx.enter_context(tc.tile_pool(name="op", bufs=4))
        pp = ctx.enter_context(tc.tile_pool(name="pp", bufs=8, space="PSUM"))

        loop = tc.For_i(0, reps, 1) if reps is not None else nullcontext()
        with loop:
            # resident x^T shard; column block t holds j=128t+p
            if c["xchunk"]:
                xchunks = []
                for t in range(NT):
                    ch = xtp.tile([128, BC], DT, name=f"xc{t}")
                    nc.scalar.dma_start(
                        ch[:], bass.AP(xt, t * 128 * BC, [[BC, 128], [1, BC]]))
                    xchunks.append(ch)
                xv = lambda t, lo, hi: xchunks[t][:, lo:hi]
            else:
                xt_res = xtp.tile([128, NT * BC], DT)
                for t in range(NT):
                    nc.scalar.dma_start(
                        xt_res[:, t * BC:(t + 1) * BC],
                        bass.AP(xt, t * 128 * BC, [[BC, 128], [1, BC]]),
                    )
                xv = lambda t, lo, hi: xt_res[:, t * BC + lo:t * BC + hi]
            bias_sb = bp.tile([128, NW], F32)
            nc.sync.dma_start(bias_sb[:], bass.AP(bias, 0, [[NW, 128], [1, NW]]))

            step = 2 if c["wpair"] else 1
            for w0 in range(0, NW, step):
                ws = list(range(w0, min(w0 + step, NW)))
                slabs, psets = [], []
                for w in ws:
                    slab = wtp.tile([128, NT * 128], DT)
                    nc.sync.dma_start(
                        slab[:],
                        bass.AP(wt, w * 128 * NT * 128,
                                [[NT * 128, 128], [1, NT * 128]]),
                    )
                    slabs.append(slab)
                    psets.append([
                        pp.tile([128, NF], F32, name=f"ps{w % 2}_{n}", tag="ps")
                        for n in range(NN)
                    ])
                for t in range(NT):
                    for slab, psums in zip(slabs, psets):
                        lhsT = slab[:, t * 128:(t + 1) * 128]
                        for n in range(NN):
                            nc.tensor.matmul(
                                psums[n][:],
                                lhsT,
                                xv(t, n * NF, n * NF + NF),
                                start=(t == 0),
                                stop=(t == NT - 1),
                            )
                for w, psums in zip(ws, psets):
                    for n in range(NN):
                        ot = op.tile([128, NF], F32)
                        nc.vector.tensor_scalar_add(ot[:], psums[n][:],
                                                    bias_sb[:, w:w + 1])
                        yeng = (nc.sync if (c["ysplit"] and n % 2 == 1)
                                else nc.scalar)
                        yeng.dma_start(
                            bass.AP(yt, w * 128 * BC + n * NF,
                                    [[BC, 128], [1, NF]]),
                            ot[:],
                        )
    nc.compile()
    return nc


def _get_program():
    key = DEFAULT_VARIANT
    if key not in _PROGRAMS:
        _PROGRAMS[key] = _build_program()
    return _PROGRAMS[key]


def _reconstruct_wt(upper_w: np.ndarray, lower_w: np.ndarray) -> np.ndarray:
    """Dense W [o, j] from the packed strict triangles (row-major fill)."""
    W = np.zeros((N, N), dtype=np.float32)
    iu = np.triu_indices(N, k=1)
    il = np.tril_indices(N, k=-1)
    W[iu] = upper_w
    W[il] = lower_w
    return W


def _shard_inputs(x, upper_w, lower_w, bias):
    c = _cfg()
    RO, RB, OC, BC, NW = c["ro"], c["rb"], c["oc"], c["bc"], c["nw"]

    x = np.asarray(x, dtype=np.float32)
    upper_w = np.asarray(upper_w, dtype=np.float32)
    lower_w = np.asarray(lower_w, dtype=np.float32)
    bias = np.asarray(bias, dtype=np.float32)

    W = _reconstruct_wt(upper_w, lower_w)

    if c["dt"] == "bfloat16":
        import ml_dtypes
        in_dt = ml_dtypes.bfloat16
    else:
        in_dt = np.float32

    wt_shards = []
    bias_shards = []
    for ob in range(RO):
        Ws = W[ob * OC:(ob + 1) * OC, :]                       # [OC o, N j]
        # wt[w, p, t, o'] = Ws[128w+o', 128t+p]
        wt = np.ascontiguousarray(
            Ws.T.reshape(NT, 128, NW, 128).transpose(2, 1, 0, 3)
        ).astype(in_dt)
        wt_shards.append(wt)
        bias_shards.append(
            np.ascontiguousarray(bias[ob * OC:(ob + 1) * OC].reshape(NW, 128).T)
        )

    xt_shards = []
    for bb in range(RB):
        xs = x[bb * BC:(bb + 1) * BC, :]                       # [BC b, N j]
        xt_shards.append(
            np.ascontiguousarray(xs.T.reshape(NT, 128, BC)).astype(in_dt)
        )

    in_maps = []
    for core in range(8):
        ob, bb = core // RB, core % RB
        in_maps.append({
            "xt": xt_shards[bb],
            "wt": wt_shards[ob],
            "bias": bias_shards[ob],
        })
    return in_maps


def _assemble(results) -> np.ndarray:
    c = _cfg()
    RB, OC, BC = c["rb"], c["oc"], c["bc"]
    y = np.empty((N, N), dtype=np.float32)
    for core in range(8):
        ob, bb = core // RB, core % RB
        y[bb * BC:(bb + 1) * BC, ob * OC:(ob + 1) * OC] = results[core]["yt"].T
    return y


def kernel(x, upper_w, lower_w, bias):
    from concourse import bass_utils

    nc = _get_program()
    in_maps = _shard_inputs(x, upper_w, lower_w, bias)
    res = bass_utils.run_bass_kernel_spmd(nc, in_maps, core_ids=list(range(8)))
    return _assemble(res.results)
